# revision 27
# baseline (speedup 1.0000x reference)
"""AtomWiseInvariants (GNN message passing) on 8 TRN2 NeuronCores.

Strategy: shard by destination node; core i owns nodes [i*N/8, (i+1)*N/8).
Within each core, nodes are ordered by degree (desc) and grouped into
128-node windows. Edge layout: the k-th edge (by rank) of the node at
window slot l lives in tile k of that window, column l. Consequently the
scatter-add is an *identity* accumulation over tiles — and since a
matmul by W1 commutes with that sum, the accumulation matmul uses W1 as
its stationary operand, computing the MLP's first layer pre-activation
directly: h1pre[c',l] += (W1 @ msgT_k)[c',l], PSUM-accumulated over the
window's tiles with a stride-0 output AP fusing same-window runs.

The filter matmul exploits K=21 << 128: four compute groups' filter
matmuls are issued to distinct 32-row PE quadrants (tile_position row
tiling) so they stream concurrently:
  filtT_j = waq[32j:32j+21].T @ rbf_env_j     (j = 0..3, one superblock)
  msgT_j  = filtT_j * xT_j                    -> DVE / ACT+DVE split

All streams bf16; env and the rbf bias row are folded on the host. Per
4-window batch the remaining 2-layer MLP runs on [C, 512] transposed
activations. All 8 cores run the same compiled graph (SPMD): per-window
tile counts are the max over cores; shortfall is zero-padded.
"""

import math

import numpy as np

# ---------------------------------------------------------------- config

NCORES = 8
P = 128            # partitions / window node count / tile edge count
RBF_DIM = 20
RK = RBF_DIM + 1   # augmented contraction dim (env/bias row)
GC = 4             # tiles per compute group (PSUM 512 f32 = 1 bank)
SB = 16            # tiles per superblock (4 groups, row-tiled quads)
GX = 16            # tiles per x DMA chunk
GR = 64            # tiles per rbf DMA chunk
ACT_ROUTE = 2      # 1 of ACT_ROUTE groups goes via ACT copy (0 = never)
FUSE_ACC = True    # fuse a group's acc matmuls via stride-0 out AP

# CoreSim lacks Silu; True decomposes it as v*sigmoid(v) for sim runs
SILU_DECOMP = False
DEBUG_DUMP = False  # extra "dbg" output with sb0 intermediates


# ------------------------------------------------------------- host prep

def prepare(x_scalar, rbf, envelop_para, edge_index_0, num_atoms,
            W_rbf, b_rbf, W1, b1, W2, b2, W3, b3):
    """Host-side sharding/layout (permutation + padding only).

    Returns (in_maps, meta)."""
    import ml_dtypes
    bf16 = ml_dtypes.bfloat16

    N = int(num_atoms)
    C = x_scalar.shape[1]
    assert N % NCORES == 0
    npc = N // NCORES
    W = math.ceil(npc / P)

    dst = np.asarray(edge_index_0, dtype=np.int64)
    order = np.argsort(dst, kind="stable")
    dst_s = dst[order]
    x_s = np.asarray(x_scalar, dtype=np.float32)[order]
    rbf_s = np.asarray(rbf, dtype=np.float32)[order]
    env_s = np.asarray(envelop_para, dtype=np.float32).reshape(-1)[order]

    deg = np.bincount(dst_s, minlength=N)
    starts = np.zeros(N, dtype=np.int64)
    starts[1:] = np.cumsum(deg)[:-1]
    rank = np.arange(len(dst_s), dtype=np.int64) - starts[dst_s]

    # per-core degree-desc node permutation; window/slot of each node
    perms = []
    win_of = np.zeros(N, dtype=np.int64)
    lid_of = np.zeros(N, dtype=np.int64)
    first_deg = np.zeros((NCORES, W), dtype=np.int64)
    for c in range(NCORES):
        lo = c * npc
        nodes = lo + np.argsort(-deg[lo:lo + npc], kind="stable")
        perms.append(nodes)
        pos = np.arange(npc, dtype=np.int64)
        win_of[nodes] = pos >> 7
        lid_of[nodes] = pos & 127
        fd = deg[nodes[::P]]
        first_deg[c, :len(fd)] = fd

    tiles_w = np.maximum(1, first_deg.max(axis=0))
    TT = int(tiles_w.sum())
    TTp = -(-TT // GR) * GR           # pad arrays to the DMA chunk lcm
    tile_off = np.zeros(W + 1, dtype=np.int64)
    np.cumsum(tiles_w, out=tile_off[1:])

    t_of_edge = tile_off[win_of[dst_s]] + rank
    flat = t_of_edge * P + lid_of[dst_s]

    core_of = dst_s // npc
    core_bounds = np.searchsorted(core_of, np.arange(NCORES + 1))

    wa = np.zeros((RK, C), dtype=np.float32)
    wa[:RBF_DIM] = np.asarray(W_rbf, np.float32).T
    wa[RBF_DIM] = np.asarray(b_rbf, np.float32)
    waq = np.zeros((P, C), dtype=np.float32)
    for j in range(4):
        waq[32 * j:32 * j + RK] = wa
    consts = {
        "waq": waq.astype(bf16),
        "w1t": np.ascontiguousarray(np.asarray(W1, np.float32).T).astype(bf16),
        "w2t": np.ascontiguousarray(np.asarray(W2, np.float32).T).astype(bf16),
        "w3t": np.ascontiguousarray(np.asarray(W3, np.float32).T).astype(bf16),
        "b1": np.asarray(b1, np.float32).reshape(C, 1),
        "b2": np.asarray(b2, np.float32).reshape(C, 1),
        "b3": np.asarray(b3, np.float32).reshape(1, 1),
    }

    NG = TTp // GC
    NSB = TTp // SB
    in_maps = []
    for c in range(NCORES):
        lo, hi = core_bounds[c], core_bounds[c + 1]
        sl = flat[lo:hi]

        Xf = np.zeros((TTp * P, C), dtype=np.float32)
        Xf[sl] = x_s[lo:hi]
        xg = (Xf.reshape(TTp, P, C).transpose(0, 2, 1)
              .reshape(TTp // GX, GX, C, P).transpose(0, 2, 1, 3)
              .reshape(TTp // GX, C, GX * P)).astype(bf16)

        Rf = np.zeros((TTp * P, RK), dtype=np.float32)
        Rf[sl, :RBF_DIM] = rbf_s[lo:hi] * env_s[lo:hi, None]
        Rf[sl, RBF_DIM] = env_s[lo:hi]
        # [TT,P,RK] -> [NG,RK,4P] -> quad-stack groups into superblocks
        Rg = (Rf.reshape(TTp, P, RK).transpose(0, 2, 1)
              .reshape(NG, GC, RK, P).transpose(0, 2, 1, 3)
              .reshape(NG, RK, GC * P)).reshape(NSB, 4, RK, GC * P)
        RS = np.zeros((NSB, P, GC * P), dtype=np.float32)
        for j in range(4):
            RS[:, 32 * j:32 * j + RK, :] = Rg[:, j]
        rbg = (RS.reshape(TTp // GR, GR // SB, P, GC * P)
               .transpose(0, 2, 1, 3)
               .reshape(TTp // GR, P, (GR // SB) * GC * P)).astype(bf16)

        in_maps.append({"xg": np.ascontiguousarray(xg),
                        "rbg": np.ascontiguousarray(rbg), **consts})

    meta = dict(N=N, C=C, npc=npc, W=W, TT=TTp, TT_real=TT,
                tiles_w=tiles_w.tolist(), perms=perms)
    return in_maps, meta


# ----------------------------------------------------------- bass kernel

def build_graph(meta):
    import concourse.bacc as bacc
    import concourse.mybir as mybir
    import concourse.tile as tile

    f32 = mybir.dt.float32
    bf16 = mybir.dt.bfloat16
    AF = mybir.ActivationFunctionType
    OP = mybir.AluOpType

    C = meta["C"]
    W = meta["W"]
    tiles_w = meta["tiles_w"]
    TT = meta["TT"]
    TTr = meta["TT_real"]

    nc = bacc.Bacc(None, target_bir_lowering=False, debug=False)

    xg_d = nc.declare_dram_parameter("xg", [TT // GX, C, GX * P], bf16,
                                     isOutput=False)
    rbg_d = nc.declare_dram_parameter("rbg", [TT // GR, P, (GR // SB) *
                                              GC * P], bf16, isOutput=False)
    waq_d = nc.declare_dram_parameter("waq", [P, C], bf16, isOutput=False)
    w1t_d = nc.declare_dram_parameter("w1t", [C, C], bf16, isOutput=False)
    w2t_d = nc.declare_dram_parameter("w2t", [C, C], bf16, isOutput=False)
    w3t_d = nc.declare_dram_parameter("w3t", [C, 1], bf16, isOutput=False)
    b1_d = nc.declare_dram_parameter("b1", [C, 1], f32, isOutput=False)
    b2_d = nc.declare_dram_parameter("b2", [C, 1], f32, isOutput=False)
    b3_d = nc.declare_dram_parameter("b3", [1, 1], f32, isOutput=False)
    out_d = nc.declare_dram_parameter("out", [W * P], f32, isOutput=True)
    if DEBUG_DUMP:
        dbg_d = nc.declare_dram_parameter("dbg", [9, C, GC * P], f32,
                                          isOutput=True)

    with tile.TileContext(nc) as tc:
        with (
            tc.tile_pool(name="const", bufs=1) as cp,
            tc.tile_pool(name="xin", bufs=3) as xp,
            tc.tile_pool(name="rin", bufs=2) as rp,
            tc.tile_pool(name="fe", bufs=3) as fep,
            tc.tile_pool(name="msg", bufs=6) as mp,
            tc.tile_pool(name="mlp", bufs=2) as hp,
            tc.tile_pool(name="fps", bufs=3, space="PSUM") as fps,
            tc.tile_pool(name="wps", bufs=2, space="PSUM") as wps,
        ):
            waq_s = cp.tile([P, C], bf16)
            nc.sync.dma_start(out=waq_s[:], in_=waq_d[:, :])
            w1t_s = cp.tile([C, C], bf16)
            nc.sync.dma_start(out=w1t_s[:], in_=w1t_d[:, :])
            w2t_s = cp.tile([C, C], bf16)
            nc.sync.dma_start(out=w2t_s[:], in_=w2t_d[:, :])
            w3t_s = cp.tile([C, 1], bf16)
            nc.sync.dma_start(out=w3t_s[:], in_=w3t_d[:, :])
            b1_s = cp.tile([C, 1], f32)
            nc.sync.dma_start(out=b1_s[:], in_=b1_d[:, :])
            b2_s = cp.tile([C, 1], f32)
            nc.sync.dma_start(out=b2_s[:], in_=b2_d[:, :])
            b3_s = cp.tile([1, 1], f32)
            nc.sync.dma_start(out=b3_s[:], in_=b3_d[:, :])
            ystrip = cp.tile([1, W * P], f32)

            def silu(h, hpsum, bias):
                if SILU_DECOMP:
                    z = hp.tile([C, 4 * P], f32, tag="siluz")
                    nc.scalar.activation(z[:, :h.shape[1]], hpsum,
                                         AF.Identity, bias=bias[:])
                    s = hp.tile([C, 4 * P], f32, tag="silus")
                    nc.scalar.activation(s[:, :h.shape[1]], hpsum,
                                         AF.Sigmoid, bias=bias[:])
                    nc.vector.tensor_tensor(out=h, in0=z[:, :h.shape[1]],
                                            in1=s[:, :h.shape[1]],
                                            op=OP.mult)
                else:
                    nc.scalar.activation(h, hpsum, AF.Silu, bias=bias[:])

            sched = [(w, k) for w in range(W) for k in range(tiles_w[w])]
            NSB = -(-(-(-TTr // GC)) // 4)  # ceil(ceil(TTr/GC)/4)
            nbat = math.ceil(W / 4)
            msgs = {}                # group -> msg4 SBUF tile
            hold = {}                # dma buffers
            outws = {}               # batch -> h1pre accumulating PSUM

            def emit_sb(sb):
                lo = sb * SB
                if lo % GR == 0:
                    rb = rp.tile([P, (GR // SB) * GC * P], bf16, tag="r",
                                 name="rb")
                    nc.sync.dma_start(out=rb[:], in_=rbg_d[lo // GR, :, :])
                    hold["rb"] = rb
                if lo % GX == 0:
                    x4 = xp.tile([C, GX * P], bf16, tag="x", name="x4")
                    nc.sync.dma_start(out=x4[:], in_=xg_d[lo // GX, :, :])
                    hold["x4"] = x4
                rb = hold["rb"]
                co = (sb % (GR // SB)) * GC * P
                nq = min(4, -(-(TTr - lo) // GC))
                filts = []
                for pr in range(2):
                    if 2 * pr >= nq:
                        break
                    filt2 = fps.tile([C, 2 * GC * P], f32, space="PSUM",
                                     name="filt")
                    filts.append(filt2)
                    for jj in range(min(2, nq - 2 * pr)):
                        j = 2 * pr + jj
                        nc.tensor.matmul(
                            out=filt2[:, jj * GC * P:(jj + 1) * GC * P],
                            lhsT=waq_s[32 * j:32 * j + RK, :],
                            rhs=rb[32 * j:32 * j + RK, co:co + GC * P],
                            start=True, stop=True,
                            tile_position=(32 * j, 0))
                for pr in range(2):
                    if 2 * pr >= nq:
                        break
                    npair = min(2, nq - 2 * pr) * GC * P
                    gp = sb * 2 + pr
                    xo = pr * 2 * GC * P
                    msg8 = mp.tile([C, 2 * GC * P], bf16, name="msg8")
                    fsrc = filts[pr]
                    half = GC * P
                    if npair > half:
                        # ACT copies the low half while DVE direct-muls
                        # the high half; both finish ~together and free
                        # the PSUM pair tile early.
                        fe = fep.tile([C, GC * P], bf16, name="fe")
                        nc.scalar.activation(fe[:], fsrc[:, :half],
                                             AF.Copy)
                        nc.vector.tensor_tensor(
                            out=msg8[:, half:npair],
                            in0=fsrc[:, half:npair],
                            in1=hold["x4"][:, xo + half:xo + npair],
                            op=OP.mult)
                        nc.vector.tensor_tensor(
                            out=msg8[:, :half], in0=fe[:],
                            in1=hold["x4"][:, xo:xo + half], op=OP.mult)
                    else:
                        nc.vector.tensor_tensor(
                            out=msg8[:, :npair], in0=fsrc[:, :npair],
                            in1=hold["x4"][:, xo:xo + npair], op=OP.mult)
                    msgs[gp] = msg8

            def emit_mlp(wb):
                outw = outws.pop(wb)
                n = (min(wb * 4 + 4, W) - wb * 4) * P
                if DEBUG_DUMP and wb == 0:
                    dtile3 = cp.tile([C, 4 * P], f32, name="dtile3")
                    nc.vector.tensor_copy(out=dtile3[:, :n],
                                          in_=outw[:, :n])
                    nc.sync.dma_start(out=dbg_d[5, :, :n], in_=dtile3[:, :n])
                h1 = hp.tile([C, 4 * P], bf16, tag="h1")
                silu(h1[:, :n], outw[:, :n], b1_s)
                if DEBUG_DUMP and wb == 0:
                    dt6 = cp.tile([C, 4 * P], f32, name="dt6")
                    nc.vector.tensor_copy(out=dt6[:, :n], in_=h1[:, :n])
                    nc.sync.dma_start(out=dbg_d[6, :, :n], in_=dt6[:, :n])
                h2p = fps.tile([C, 2 * GC * P], f32, space="PSUM",
                               name="filt")
                nc.tensor.matmul(out=h2p[:, :n], lhsT=w2t_s[:],
                                 rhs=h1[:, :n], start=True, stop=True)
                h2 = hp.tile([C, 4 * P], bf16, tag="h2")
                silu(h2[:, :n], h2p[:, :n], b2_s)
                if DEBUG_DUMP and wb == 0:
                    dt7 = cp.tile([C, 4 * P], f32, name="dt7")
                    nc.vector.tensor_copy(out=dt7[:, :n], in_=h2[:, :n])
                    nc.sync.dma_start(out=dbg_d[7, :, :n], in_=dt7[:, :n])
                    dt8 = cp.tile([C, 4 * P], f32, name="dt8")
                    nc.vector.tensor_copy(out=dt8[:, :n], in_=h2p[:, :n])
                    nc.sync.dma_start(out=dbg_d[8, :, :n], in_=dt8[:, :n])
                nc.tensor.matmul(out=h2p[0:1, :n], lhsT=w3t_s[:],
                                 rhs=h2[:, :n], start=True, stop=True)
                nc.scalar.activation(
                    ystrip[:, wb * 4 * P:wb * 4 * P + n], h2p[0:1, :n],
                    AF.Identity, bias=b3_s[:])

            def emit_acc(gp):
                if gp not in msgs:
                    return
                msg8 = msgs.pop(gp)
                lo = gp * 2 * GC
                nreal = min(2 * GC, TTr - lo)
                j = 0
                while j < nreal:
                    w, k = sched[lo + j]
                    wb = w // 4
                    if wb not in outws:
                        outws[wb] = wps.tile([C, 4 * P], f32, space="PSUM",
                                             name="outw")
                    outw = outws[wb]
                    run = 1
                    while (run < 4 and j + run < nreal
                           and sched[lo + j + run][0] == w):
                        run += 1
                    klast = k + run - 1
                    reg = outw[:, (w % 4) * P:(w % 4 + 1) * P]
                    if FUSE_ACC and run > 1:
                        nc.tensor.matmul(
                            out=reg.unsqueeze(1).broadcast_to([C, run, P]),
                            lhsT=w1t_s[:],
                            rhs=msg8[:, j * P:(j + run) * P],
                            start=(k == 0),
                            stop=(klast == tiles_w[w] - 1))
                    else:
                        for q in range(run):
                            nc.tensor.matmul(
                                out=reg, lhsT=w1t_s[:],
                                rhs=msg8[:, (j + q) * P:(j + q + 1) * P],
                                start=(k + q == 0),
                                stop=(k + q == tiles_w[w] - 1))
                    if klast == tiles_w[w] - 1 and (w % 4 == 3 or w == W - 1):
                        emit_mlp(wb)
                    j += run

            for sb in range(NSB + 1):
                if sb < NSB:
                    emit_sb(sb)
                if sb >= 1:
                    for pr in range(2):
                        emit_acc((sb - 1) * 2 + pr)

            nc.sync.dma_start(out=out_d[None, :], in_=ystrip[:])

    nc.compile()
    return nc


# --------------------------------------------------------------- driver

def run(inputs, trace=False, tmpdir=None):
    from concourse.bass_utils import run_bass_kernel_spmd

    in_maps, meta = prepare(**inputs)
    nc = build_graph(meta)
    res = run_bass_kernel_spmd(nc, in_maps, core_ids=list(range(NCORES)),
                               trace=trace, tmpdir=tmpdir)
    npc = meta["npc"]
    N = meta["N"]
    out = np.zeros(N, dtype=np.float32)
    for c in range(NCORES):
        out[meta["perms"][c]] = res.results[c]["out"][:npc]
    return out.reshape(N, 1), res


def kernel(**inputs):
    out, _ = run(inputs, trace=False)
    return out


# revision 28
# speedup vs baseline: 1.0349x; 1.0349x over previous
"""AtomWiseInvariants (GNN message passing) on 8 TRN2 NeuronCores.

Strategy: shard by destination node; core i owns nodes [i*N/8, (i+1)*N/8).
Within each core, nodes are ordered by degree (desc) and grouped into
128-node windows. Edge layout: the k-th edge (by rank) of the node at
window slot l lives in tile k of that window, column l. Consequently the
scatter-add is an *identity* accumulation over tiles — and since a
matmul by W1 commutes with that sum, the accumulation matmul uses W1 as
its stationary operand, computing the MLP's first layer pre-activation
directly: h1pre[c',l] += (W1 @ msgT_k)[c',l], PSUM-accumulated over the
window's tiles with a stride-0 output AP fusing same-window runs.

The filter matmul exploits K=21 << 128: four compute groups' filter
matmuls are issued to distinct 32-row PE quadrants (tile_position row
tiling) so they stream concurrently:
  filtT_j = waq[32j:32j+21].T @ rbf_env_j     (j = 0..3, one superblock)
  msgT_j  = filtT_j * xT_j                    -> DVE / ACT+DVE split

All streams bf16; env and the rbf bias row are folded on the host. Per
4-window batch the remaining 2-layer MLP runs on [C, 512] transposed
activations. All 8 cores run the same compiled graph (SPMD): per-window
tile counts are the max over cores; shortfall is zero-padded.
"""

import math

import numpy as np

# ---------------------------------------------------------------- config

NCORES = 8
P = 128            # partitions / window node count / tile edge count
RBF_DIM = 20
RK = RBF_DIM + 1   # augmented contraction dim (env/bias row)
GC = 4             # tiles per compute group (PSUM 512 f32 = 1 bank)
SB = 16            # tiles per superblock (4 groups, row-tiled quads)
GX = 16            # tiles per x DMA chunk
GR = 64            # tiles per rbf DMA chunk
ACT_ROUTE = 2      # 1 of ACT_ROUTE groups goes via ACT copy (0 = never)
FUSE_ACC = True    # fuse a group's acc matmuls via stride-0 out AP

# CoreSim lacks Silu; True decomposes it as v*sigmoid(v) for sim runs
SILU_DECOMP = False
DEBUG_DUMP = False  # extra "dbg" output with sb0 intermediates


# ------------------------------------------------------------- host prep

def prepare(x_scalar, rbf, envelop_para, edge_index_0, num_atoms,
            W_rbf, b_rbf, W1, b1, W2, b2, W3, b3):
    """Host-side sharding/layout (permutation + padding only).

    Returns (in_maps, meta)."""
    import ml_dtypes
    bf16 = ml_dtypes.bfloat16

    N = int(num_atoms)
    C = x_scalar.shape[1]
    assert N % NCORES == 0
    npc = N // NCORES
    W = math.ceil(npc / P)

    dst = np.asarray(edge_index_0, dtype=np.int64)
    order = np.argsort(dst, kind="stable")
    dst_s = dst[order]
    x_s = np.asarray(x_scalar, dtype=np.float32)[order]
    rbf_s = np.asarray(rbf, dtype=np.float32)[order]
    env_s = np.asarray(envelop_para, dtype=np.float32).reshape(-1)[order]

    deg = np.bincount(dst_s, minlength=N)
    starts = np.zeros(N, dtype=np.int64)
    starts[1:] = np.cumsum(deg)[:-1]
    rank = np.arange(len(dst_s), dtype=np.int64) - starts[dst_s]

    # per-core degree-desc node permutation; window/slot of each node
    perms = []
    win_of = np.zeros(N, dtype=np.int64)
    lid_of = np.zeros(N, dtype=np.int64)
    first_deg = np.zeros((NCORES, W), dtype=np.int64)
    for c in range(NCORES):
        lo = c * npc
        nodes = lo + np.argsort(-deg[lo:lo + npc], kind="stable")
        perms.append(nodes)
        pos = np.arange(npc, dtype=np.int64)
        win_of[nodes] = pos >> 7
        lid_of[nodes] = pos & 127
        fd = deg[nodes[::P]]
        first_deg[c, :len(fd)] = fd

    tiles_w = np.maximum(1, first_deg.max(axis=0))
    TT = int(tiles_w.sum())
    TTp = -(-TT // GR) * GR           # pad arrays to the DMA chunk lcm
    tile_off = np.zeros(W + 1, dtype=np.int64)
    np.cumsum(tiles_w, out=tile_off[1:])

    t_of_edge = tile_off[win_of[dst_s]] + rank
    flat = t_of_edge * P + lid_of[dst_s]

    core_of = dst_s // npc
    core_bounds = np.searchsorted(core_of, np.arange(NCORES + 1))

    wa = np.zeros((RK, C), dtype=np.float32)
    wa[:RBF_DIM] = np.asarray(W_rbf, np.float32).T
    wa[RBF_DIM] = np.asarray(b_rbf, np.float32)
    waq = np.zeros((P, C), dtype=np.float32)
    for j in range(4):
        waq[32 * j:32 * j + RK] = wa
    consts = {
        "waq": waq.astype(bf16),
        "w1t": np.ascontiguousarray(np.asarray(W1, np.float32).T).astype(bf16),
        "w2t": np.ascontiguousarray(np.asarray(W2, np.float32).T).astype(bf16),
        "w3t": np.ascontiguousarray(np.asarray(W3, np.float32).T).astype(bf16),
        "b1": np.asarray(b1, np.float32).reshape(C, 1),
        "b2": np.asarray(b2, np.float32).reshape(C, 1),
        "b3": np.asarray(b3, np.float32).reshape(1, 1),
    }

    NG = TTp // GC
    NSB = TTp // SB
    in_maps = []
    for c in range(NCORES):
        lo, hi = core_bounds[c], core_bounds[c + 1]
        sl = flat[lo:hi]

        Xf = np.zeros((TTp * P, C), dtype=np.float32)
        Xf[sl] = x_s[lo:hi]
        xg = (Xf.reshape(TTp, P, C).transpose(0, 2, 1)
              .reshape(TTp // GX, GX, C, P).transpose(0, 2, 1, 3)
              .reshape(TTp // GX, C, GX * P)).astype(bf16)

        Rf = np.zeros((TTp * P, RK), dtype=np.float32)
        Rf[sl, :RBF_DIM] = rbf_s[lo:hi] * env_s[lo:hi, None]
        Rf[sl, RBF_DIM] = env_s[lo:hi]
        # [TT,P,RK] -> [NG,RK,4P] -> quad-stack groups into superblocks
        Rg = (Rf.reshape(TTp, P, RK).transpose(0, 2, 1)
              .reshape(NG, GC, RK, P).transpose(0, 2, 1, 3)
              .reshape(NG, RK, GC * P)).reshape(NSB, 4, RK, GC * P)
        RS = np.zeros((NSB, P, GC * P), dtype=np.float32)
        for j in range(4):
            RS[:, 32 * j:32 * j + RK, :] = Rg[:, j]
        rbg = (RS.reshape(TTp // GR, GR // SB, P, GC * P)
               .transpose(0, 2, 1, 3)
               .reshape(TTp // GR, P, (GR // SB) * GC * P)).astype(bf16)

        in_maps.append({"xg": np.ascontiguousarray(xg),
                        "rbg": np.ascontiguousarray(rbg), **consts})

    meta = dict(N=N, C=C, npc=npc, W=W, TT=TTp, TT_real=TT,
                tiles_w=tiles_w.tolist(), perms=perms)
    return in_maps, meta


# ----------------------------------------------------------- bass kernel

def build_graph(meta):
    import concourse.bacc as bacc
    import concourse.mybir as mybir
    import concourse.tile as tile

    f32 = mybir.dt.float32
    bf16 = mybir.dt.bfloat16
    AF = mybir.ActivationFunctionType
    OP = mybir.AluOpType

    C = meta["C"]
    W = meta["W"]
    tiles_w = meta["tiles_w"]
    TT = meta["TT"]
    TTr = meta["TT_real"]

    nc = bacc.Bacc(None, target_bir_lowering=False, debug=False)

    xg_d = nc.declare_dram_parameter("xg", [TT // GX, C, GX * P], bf16,
                                     isOutput=False)
    rbg_d = nc.declare_dram_parameter("rbg", [TT // GR, P, (GR // SB) *
                                              GC * P], bf16, isOutput=False)
    waq_d = nc.declare_dram_parameter("waq", [P, C], bf16, isOutput=False)
    w1t_d = nc.declare_dram_parameter("w1t", [C, C], bf16, isOutput=False)
    w2t_d = nc.declare_dram_parameter("w2t", [C, C], bf16, isOutput=False)
    w3t_d = nc.declare_dram_parameter("w3t", [C, 1], bf16, isOutput=False)
    b1_d = nc.declare_dram_parameter("b1", [C, 1], f32, isOutput=False)
    b2_d = nc.declare_dram_parameter("b2", [C, 1], f32, isOutput=False)
    b3_d = nc.declare_dram_parameter("b3", [1, 1], f32, isOutput=False)
    out_d = nc.declare_dram_parameter("out", [W * P], f32, isOutput=True)
    if DEBUG_DUMP:
        dbg_d = nc.declare_dram_parameter("dbg", [9, C, GC * P], f32,
                                          isOutput=True)

    with tile.TileContext(nc) as tc:
        with (
            tc.tile_pool(name="const", bufs=1) as cp,
            tc.tile_pool(name="xin", bufs=3) as xp,
            tc.tile_pool(name="rin", bufs=2) as rp,
            tc.tile_pool(name="fe", bufs=3) as fep,
            tc.tile_pool(name="msg", bufs=6) as mp,
            tc.tile_pool(name="mlp", bufs=2) as hp,
            tc.tile_pool(name="fps", bufs=3, space="PSUM") as fps,
            tc.tile_pool(name="wps", bufs=2, space="PSUM") as wps,
        ):
            waq_s = cp.tile([P, C], bf16)
            nc.sync.dma_start(out=waq_s[:], in_=waq_d[:, :])
            w1t_s = cp.tile([C, C], bf16)
            nc.sync.dma_start(out=w1t_s[:], in_=w1t_d[:, :])
            w2t_s = cp.tile([C, C], bf16)
            nc.sync.dma_start(out=w2t_s[:], in_=w2t_d[:, :])
            w3t_s = cp.tile([C, 1], bf16)
            nc.sync.dma_start(out=w3t_s[:], in_=w3t_d[:, :])
            b1_s = cp.tile([C, 1], f32)
            nc.sync.dma_start(out=b1_s[:], in_=b1_d[:, :])
            b2_s = cp.tile([C, 1], f32)
            nc.sync.dma_start(out=b2_s[:], in_=b2_d[:, :])
            b3_s = cp.tile([1, 1], f32)
            nc.sync.dma_start(out=b3_s[:], in_=b3_d[:, :])
            ystrip = cp.tile([1, W * P], f32)

            def silu(h, hpsum, bias):
                if SILU_DECOMP:
                    z = hp.tile([C, 4 * P], f32, tag="siluz")
                    nc.scalar.activation(z[:, :h.shape[1]], hpsum,
                                         AF.Identity, bias=bias[:])
                    s = hp.tile([C, 4 * P], f32, tag="silus")
                    nc.scalar.activation(s[:, :h.shape[1]], hpsum,
                                         AF.Sigmoid, bias=bias[:])
                    nc.vector.tensor_tensor(out=h, in0=z[:, :h.shape[1]],
                                            in1=s[:, :h.shape[1]],
                                            op=OP.mult)
                else:
                    nc.scalar.activation(h, hpsum, AF.Silu, bias=bias[:])

            sched = [(w, k) for w in range(W) for k in range(tiles_w[w])]
            NSB = -(-(-(-TTr // GC)) // 4)  # ceil(ceil(TTr/GC)/4)
            nbat = math.ceil(W / 4)
            msgs = {}                # group -> msg4 SBUF tile
            hold = {}                # dma buffers
            outws = {}               # batch -> h1pre accumulating PSUM

            def emit_sb(sb):
                lo = sb * SB
                if lo % GR == 0:
                    rb = rp.tile([P, (GR // SB) * GC * P], bf16, tag="r",
                                 name="rb")
                    nc.sync.dma_start(out=rb[:], in_=rbg_d[lo // GR, :, :])
                    hold["rb"] = rb
                if lo % GX == 0:
                    x4 = xp.tile([C, GX * P], bf16, tag="x", name="x4")
                    nc.sync.dma_start(out=x4[:], in_=xg_d[lo // GX, :, :])
                    hold["x4"] = x4
                rb = hold["rb"]
                co = (sb % (GR // SB)) * GC * P
                nq = min(4, -(-(TTr - lo) // GC))
                filts = []
                for pr in range(2):
                    if 2 * pr >= nq:
                        break
                    filt2 = fps.tile([C, 2 * GC * P], f32, space="PSUM",
                                     name="filt")
                    filts.append(filt2)
                    for jj in range(min(2, nq - 2 * pr)):
                        j = 2 * pr + jj
                        nc.tensor.matmul(
                            out=filt2[:, jj * GC * P:(jj + 1) * GC * P],
                            lhsT=waq_s[32 * j:32 * j + RK, :],
                            rhs=rb[32 * j:32 * j + RK, co:co + GC * P],
                            start=True, stop=True,
                            tile_position=(32 * j, 0))
                for pr in range(2):
                    if 2 * pr >= nq:
                        break
                    npair = min(2, nq - 2 * pr) * GC * P
                    gp = sb * 2 + pr
                    xo = pr * 2 * GC * P
                    msg8 = mp.tile([C, 2 * GC * P], bf16, name="msg8")
                    fsrc = filts[pr]
                    half = GC * P
                    if npair > half:
                        # ACT copies the low half while DVE direct-muls
                        # the high half; both finish ~together and free
                        # the PSUM pair tile early.
                        fe = fep.tile([C, GC * P], bf16, name="fe")
                        nc.scalar.activation(fe[:], fsrc[:, :half],
                                             AF.Copy)
                        nc.vector.tensor_tensor(
                            out=msg8[:, half:npair],
                            in0=fsrc[:, half:npair],
                            in1=hold["x4"][:, xo + half:xo + npair],
                            op=OP.mult)
                        nc.vector.tensor_tensor(
                            out=msg8[:, :half], in0=fe[:],
                            in1=hold["x4"][:, xo:xo + half], op=OP.mult)
                    else:
                        nc.vector.tensor_tensor(
                            out=msg8[:, :npair], in0=fsrc[:, :npair],
                            in1=hold["x4"][:, xo:xo + npair], op=OP.mult)
                    msgs[gp] = msg8

            def emit_mlp(wb):
                outw = outws.pop(wb)
                n = (min(wb * 4 + 4, W) - wb * 4) * P
                if DEBUG_DUMP and wb == 0:
                    dtile3 = cp.tile([C, 4 * P], f32, name="dtile3")
                    nc.vector.tensor_copy(out=dtile3[:, :n],
                                          in_=outw[:, :n])
                    nc.sync.dma_start(out=dbg_d[5, :, :n], in_=dtile3[:, :n])
                h1 = hp.tile([C, 4 * P], bf16, tag="h1")
                silu(h1[:, :n], outw[:, :n], b1_s)
                if DEBUG_DUMP and wb == 0:
                    dt6 = cp.tile([C, 4 * P], f32, name="dt6")
                    nc.vector.tensor_copy(out=dt6[:, :n], in_=h1[:, :n])
                    nc.sync.dma_start(out=dbg_d[6, :, :n], in_=dt6[:, :n])
                h2p = wps.tile([C, 4 * P], f32, space="PSUM",
                               name="outw")
                nc.tensor.matmul(out=h2p[:, :n], lhsT=w2t_s[:],
                                 rhs=h1[:, :n], start=True, stop=True)
                h2 = hp.tile([C, 4 * P], bf16, tag="h2")
                silu(h2[:, :n], h2p[:, :n], b2_s)
                if DEBUG_DUMP and wb == 0:
                    dt7 = cp.tile([C, 4 * P], f32, name="dt7")
                    nc.vector.tensor_copy(out=dt7[:, :n], in_=h2[:, :n])
                    nc.sync.dma_start(out=dbg_d[7, :, :n], in_=dt7[:, :n])
                    dt8 = cp.tile([C, 4 * P], f32, name="dt8")
                    nc.vector.tensor_copy(out=dt8[:, :n], in_=h2p[:, :n])
                    nc.sync.dma_start(out=dbg_d[8, :, :n], in_=dt8[:, :n])
                nc.tensor.matmul(out=h2p[0:1, :n], lhsT=w3t_s[:],
                                 rhs=h2[:, :n], start=True, stop=True)
                nc.scalar.activation(
                    ystrip[:, wb * 4 * P:wb * 4 * P + n], h2p[0:1, :n],
                    AF.Identity, bias=b3_s[:])
                nc.sync.dma_start(
                    out=out_d[None, wb * 4 * P:wb * 4 * P + n],
                    in_=ystrip[:, wb * 4 * P:wb * 4 * P + n])

            def emit_acc(gp):
                if gp not in msgs:
                    return
                msg8 = msgs.pop(gp)
                lo = gp * 2 * GC
                nreal = min(2 * GC, TTr - lo)
                j = 0
                while j < nreal:
                    w, k = sched[lo + j]
                    wb = w // 4
                    if wb not in outws:
                        outws[wb] = wps.tile([C, 4 * P], f32, space="PSUM",
                                             name="outw")
                    outw = outws[wb]
                    run = 1
                    while (run < 4 and j + run < nreal
                           and sched[lo + j + run][0] == w):
                        run += 1
                    klast = k + run - 1
                    reg = outw[:, (w % 4) * P:(w % 4 + 1) * P]
                    if FUSE_ACC and run > 1:
                        nc.tensor.matmul(
                            out=reg.unsqueeze(1).broadcast_to([C, run, P]),
                            lhsT=w1t_s[:],
                            rhs=msg8[:, j * P:(j + run) * P],
                            start=(k == 0),
                            stop=(klast == tiles_w[w] - 1))
                    else:
                        for q in range(run):
                            nc.tensor.matmul(
                                out=reg, lhsT=w1t_s[:],
                                rhs=msg8[:, (j + q) * P:(j + q + 1) * P],
                                start=(k + q == 0),
                                stop=(k + q == tiles_w[w] - 1))
                    if klast == tiles_w[w] - 1 and (w % 4 == 3 or w == W - 1):
                        emit_mlp(wb)
                    j += run

            for sb in range(NSB + 1):
                if sb < NSB:
                    emit_sb(sb)
                if sb >= 1:
                    for pr in range(2):
                        emit_acc((sb - 1) * 2 + pr)


    nc.compile()
    return nc


# --------------------------------------------------------------- driver

def run(inputs, trace=False, tmpdir=None):
    from concourse.bass_utils import run_bass_kernel_spmd

    in_maps, meta = prepare(**inputs)
    nc = build_graph(meta)
    res = run_bass_kernel_spmd(nc, in_maps, core_ids=list(range(NCORES)),
                               trace=trace, tmpdir=tmpdir)
    npc = meta["npc"]
    N = meta["N"]
    out = np.zeros(N, dtype=np.float32)
    for c in range(NCORES):
        out[meta["perms"][c]] = res.results[c]["out"][:npc]
    return out.reshape(N, 1), res


def kernel(**inputs):
    out, _ = run(inputs, trace=False)
    return out


# revision 29
# speedup vs baseline: 1.2034x; 1.1629x over previous
"""AtomWiseInvariants (GNN message passing) on 8 TRN2 NeuronCores.

Strategy: shard by destination node; core i owns nodes [i*N/8, (i+1)*N/8).
Within each core, nodes are ordered by degree (desc) and grouped into
128-node windows. Edge layout: the k-th edge (by rank) of the node at
window slot l lives in tile k of that window, column l. Consequently the
scatter-add is an *identity* accumulation over tiles — and since a
matmul by W1 commutes with that sum, the accumulation matmul uses W1 as
its stationary operand, computing the MLP's first layer pre-activation
directly: h1pre[c',l] += (W1 @ msgT_k)[c',l], PSUM-accumulated over the
window's tiles with a stride-0 output AP fusing same-window runs.

The filter matmul exploits K=21 << 128: four compute groups' filter
matmuls are issued to distinct 32-row PE quadrants (tile_position row
tiling) so they stream concurrently:
  filtT_j = waq[32j:32j+21].T @ rbf_env_j     (j = 0..3, one superblock)
  msgT_j  = filtT_j * xT_j                    -> DVE / ACT+DVE split

All streams bf16; env and the rbf bias row are folded on the host. Per
4-window batch the remaining 2-layer MLP runs on [C, 512] transposed
activations. All 8 cores run the same compiled graph (SPMD): per-window
tile counts are the max over cores; shortfall is zero-padded.
"""

import math

import numpy as np

# ---------------------------------------------------------------- config

NCORES = 8
P = 128            # partitions / window node count / tile edge count
RBF_DIM = 20
RK = RBF_DIM + 1   # augmented contraction dim (env/bias row)
GC = 4             # tiles per compute group (PSUM 512 f32 = 1 bank)
SB = 16            # tiles per superblock (4 groups, row-tiled quads)
GX = 16            # tiles per x DMA chunk
GR = 64            # tiles per rbf DMA chunk
ACT_ROUTE = 2      # 1 of ACT_ROUTE groups goes via ACT copy (0 = never)
FUSE_ACC = True    # fuse a group's acc matmuls via stride-0 out AP

# CoreSim lacks Silu; True decomposes it as v*sigmoid(v) for sim runs
SILU_DECOMP = False
DEBUG_DUMP = False  # extra "dbg" output with sb0 intermediates


# ------------------------------------------------------------- host prep

def prepare(x_scalar, rbf, envelop_para, edge_index_0, num_atoms,
            W_rbf, b_rbf, W1, b1, W2, b2, W3, b3):
    """Host-side sharding/layout (permutation + padding only).

    Returns (in_maps, meta)."""
    import ml_dtypes
    bf16 = ml_dtypes.bfloat16

    N = int(num_atoms)
    C = x_scalar.shape[1]
    assert N % NCORES == 0
    npc = N // NCORES
    W = math.ceil(npc / P)

    dst = np.asarray(edge_index_0, dtype=np.int64)
    order = np.argsort(dst, kind="stable")
    dst_s = dst[order]
    x_s = np.asarray(x_scalar, dtype=np.float32)[order]
    rbf_s = np.asarray(rbf, dtype=np.float32)[order]
    env_s = np.asarray(envelop_para, dtype=np.float32).reshape(-1)[order]

    deg = np.bincount(dst_s, minlength=N)
    starts = np.zeros(N, dtype=np.int64)
    starts[1:] = np.cumsum(deg)[:-1]
    rank = np.arange(len(dst_s), dtype=np.int64) - starts[dst_s]

    # per-core degree-desc node permutation; window/slot of each node
    perms = []
    win_of = np.zeros(N, dtype=np.int64)
    lid_of = np.zeros(N, dtype=np.int64)
    first_deg = np.zeros((NCORES, W), dtype=np.int64)
    for c in range(NCORES):
        lo = c * npc
        nodes = lo + np.argsort(-deg[lo:lo + npc], kind="stable")
        perms.append(nodes)
        pos = np.arange(npc, dtype=np.int64)
        win_of[nodes] = pos >> 7
        lid_of[nodes] = pos & 127
        fd = deg[nodes[::P]]
        first_deg[c, :len(fd)] = fd

    tiles_w = np.maximum(1, first_deg.max(axis=0))
    TT = int(tiles_w.sum())
    TTp = -(-TT // GR) * GR           # pad arrays to the DMA chunk lcm
    tile_off = np.zeros(W + 1, dtype=np.int64)
    np.cumsum(tiles_w, out=tile_off[1:])

    t_of_edge = tile_off[win_of[dst_s]] + rank
    flat = t_of_edge * P + lid_of[dst_s]

    core_of = dst_s // npc
    core_bounds = np.searchsorted(core_of, np.arange(NCORES + 1))

    wa = np.zeros((RK, C), dtype=np.float32)
    wa[:RBF_DIM] = np.asarray(W_rbf, np.float32).T
    wa[RBF_DIM] = np.asarray(b_rbf, np.float32)
    waq = np.zeros((P, C), dtype=np.float32)
    for j in range(4):
        waq[32 * j:32 * j + RK] = wa
    consts = {
        "waq": waq.astype(bf16),
        "w1t": np.ascontiguousarray(np.asarray(W1, np.float32).T).astype(bf16),
        "w2t": np.ascontiguousarray(np.asarray(W2, np.float32).T).astype(bf16),
        "w3t": np.ascontiguousarray(np.asarray(W3, np.float32).T).astype(bf16),
        "b1": np.asarray(b1, np.float32).reshape(C, 1),
        "b2": np.asarray(b2, np.float32).reshape(C, 1),
        "b3": np.asarray(b3, np.float32).reshape(1, 1),
    }

    NG = TTp // GC
    NSB = TTp // SB
    in_maps = []
    for c in range(NCORES):
        lo, hi = core_bounds[c], core_bounds[c + 1]
        sl = flat[lo:hi]

        Xf = np.zeros((TTp * P, C), dtype=np.float32)
        Xf[sl] = x_s[lo:hi]
        xg = (Xf.reshape(TTp, P, C).transpose(0, 2, 1)
              .reshape(TTp // GX, GX, C, P).transpose(0, 2, 1, 3)
              .reshape(TTp // GX, C, GX * P)).astype(bf16)

        Rf = np.zeros((TTp * P, RK), dtype=np.float32)
        Rf[sl, :RBF_DIM] = rbf_s[lo:hi] * env_s[lo:hi, None]
        Rf[sl, RBF_DIM] = env_s[lo:hi]
        # [TT,P,RK] -> [NG,RK,4P] -> quad-stack groups into superblocks
        Rg = (Rf.reshape(TTp, P, RK).transpose(0, 2, 1)
              .reshape(NG, GC, RK, P).transpose(0, 2, 1, 3)
              .reshape(NG, RK, GC * P)).reshape(NSB, 4, RK, GC * P)
        RS = np.zeros((NSB, P, GC * P), dtype=np.float32)
        for j in range(4):
            RS[:, 32 * j:32 * j + RK, :] = Rg[:, j]
        rbg = (RS.reshape(TTp // GR, GR // SB, P, GC * P)
               .transpose(0, 2, 1, 3)
               .reshape(TTp // GR, P, (GR // SB) * GC * P)).astype(bf16)

        in_maps.append({"xg": np.ascontiguousarray(xg),
                        "rbg": np.ascontiguousarray(rbg), **consts})

    meta = dict(N=N, C=C, npc=npc, W=W, TT=TTp, TT_real=TT,
                tiles_w=tiles_w.tolist(), perms=perms)
    return in_maps, meta


# ----------------------------------------------------------- bass kernel

def build_graph(meta):
    import concourse.bacc as bacc
    import concourse.mybir as mybir
    import concourse.tile as tile

    f32 = mybir.dt.float32
    bf16 = mybir.dt.bfloat16
    AF = mybir.ActivationFunctionType
    OP = mybir.AluOpType

    C = meta["C"]
    W = meta["W"]
    tiles_w = meta["tiles_w"]
    TT = meta["TT"]
    TTr = meta["TT_real"]

    nc = bacc.Bacc(None, target_bir_lowering=False, debug=False)

    xg_d = nc.declare_dram_parameter("xg", [TT // GX, C, GX * P], bf16,
                                     isOutput=False)
    rbg_d = nc.declare_dram_parameter("rbg", [TT // GR, P, (GR // SB) *
                                              GC * P], bf16, isOutput=False)
    waq_d = nc.declare_dram_parameter("waq", [P, C], bf16, isOutput=False)
    w1t_d = nc.declare_dram_parameter("w1t", [C, C], bf16, isOutput=False)
    w2t_d = nc.declare_dram_parameter("w2t", [C, C], bf16, isOutput=False)
    w3t_d = nc.declare_dram_parameter("w3t", [C, 1], bf16, isOutput=False)
    b1_d = nc.declare_dram_parameter("b1", [C, 1], f32, isOutput=False)
    b2_d = nc.declare_dram_parameter("b2", [C, 1], f32, isOutput=False)
    b3_d = nc.declare_dram_parameter("b3", [1, 1], f32, isOutput=False)
    out_d = nc.declare_dram_parameter("out", [W * P], f32, isOutput=True)
    if DEBUG_DUMP:
        dbg_d = nc.declare_dram_parameter("dbg", [9, C, GC * P], f32,
                                          isOutput=True)

    with tile.TileContext(nc) as tc:
        with (
            tc.tile_pool(name="const", bufs=1) as cp,
            tc.tile_pool(name="xin", bufs=4) as xp,
            tc.tile_pool(name="rin", bufs=3) as rp,
            tc.tile_pool(name="fe", bufs=5) as fep,
            tc.tile_pool(name="msg", bufs=8) as mp,
            tc.tile_pool(name="mlp", bufs=2) as hp,
            tc.tile_pool(name="fps", bufs=3, space="PSUM") as fps,
            tc.tile_pool(name="wps", bufs=2, space="PSUM") as wps,
        ):
            waq_s = cp.tile([P, C], bf16)
            nc.sync.dma_start(out=waq_s[:], in_=waq_d[:, :])
            w1t_s = cp.tile([C, C], bf16)
            nc.sync.dma_start(out=w1t_s[:], in_=w1t_d[:, :])
            w2t_s = cp.tile([C, C], bf16)
            nc.sync.dma_start(out=w2t_s[:], in_=w2t_d[:, :])
            w3t_s = cp.tile([C, 1], bf16)
            nc.sync.dma_start(out=w3t_s[:], in_=w3t_d[:, :])
            b1_s = cp.tile([C, 1], f32)
            nc.sync.dma_start(out=b1_s[:], in_=b1_d[:, :])
            b2_s = cp.tile([C, 1], f32)
            nc.sync.dma_start(out=b2_s[:], in_=b2_d[:, :])
            b3_s = cp.tile([1, 1], f32)
            nc.sync.dma_start(out=b3_s[:], in_=b3_d[:, :])
            ystrip = cp.tile([1, W * P], f32)

            def silu(h, hpsum, bias):
                if SILU_DECOMP:
                    z = hp.tile([C, 4 * P], f32, tag="siluz")
                    nc.scalar.activation(z[:, :h.shape[1]], hpsum,
                                         AF.Identity, bias=bias[:])
                    s = hp.tile([C, 4 * P], f32, tag="silus")
                    nc.scalar.activation(s[:, :h.shape[1]], hpsum,
                                         AF.Sigmoid, bias=bias[:])
                    nc.vector.tensor_tensor(out=h, in0=z[:, :h.shape[1]],
                                            in1=s[:, :h.shape[1]],
                                            op=OP.mult)
                else:
                    nc.scalar.activation(h, hpsum, AF.Silu, bias=bias[:])

            sched = [(w, k) for w in range(W) for k in range(tiles_w[w])]
            NSB = -(-(-(-TTr // GC)) // 4)  # ceil(ceil(TTr/GC)/4)
            nbat = math.ceil(W / 4)
            msgs = {}                # group -> msg4 SBUF tile
            hold = {}                # dma buffers
            outws = {}               # batch -> h1pre accumulating PSUM

            def emit_sb(sb):
                lo = sb * SB
                if lo % GR == 0:
                    rb = rp.tile([P, (GR // SB) * GC * P], bf16, tag="r",
                                 name="rb")
                    nc.sync.dma_start(out=rb[:], in_=rbg_d[lo // GR, :, :])
                    hold["rb"] = rb
                if lo % GX == 0:
                    x4 = xp.tile([C, GX * P], bf16, tag="x", name="x4")
                    nc.sync.dma_start(out=x4[:], in_=xg_d[lo // GX, :, :])
                    hold["x4"] = x4
                rb = hold["rb"]
                co = (sb % (GR // SB)) * GC * P
                nq = min(4, -(-(TTr - lo) // GC))
                filts = []
                for pr in range(2):
                    if 2 * pr >= nq:
                        break
                    filt2 = fps.tile([C, 2 * GC * P], f32, space="PSUM",
                                     name="filt")
                    filts.append(filt2)
                    for jj in range(min(2, nq - 2 * pr)):
                        j = 2 * pr + jj
                        nc.tensor.matmul(
                            out=filt2[:, jj * GC * P:(jj + 1) * GC * P],
                            lhsT=waq_s[32 * j:32 * j + RK, :],
                            rhs=rb[32 * j:32 * j + RK, co:co + GC * P],
                            start=True, stop=True,
                            tile_position=(32 * j, 0))
                for pr in range(2):
                    if 2 * pr >= nq:
                        break
                    npair = min(2, nq - 2 * pr) * GC * P
                    gp = sb * 2 + pr
                    xo = pr * 2 * GC * P
                    msg8 = mp.tile([C, 2 * GC * P], bf16, name="msg8")
                    fsrc = filts[pr]
                    half = GC * P
                    if npair > half:
                        # ACT copies the low half while DVE direct-muls
                        # the high half; both finish ~together and free
                        # the PSUM pair tile early.
                        fe = fep.tile([C, GC * P], bf16, name="fe")
                        nc.scalar.activation(fe[:], fsrc[:, :half],
                                             AF.Copy)
                        nc.vector.tensor_tensor(
                            out=msg8[:, half:npair],
                            in0=fsrc[:, half:npair],
                            in1=hold["x4"][:, xo + half:xo + npair],
                            op=OP.mult)
                        nc.vector.tensor_tensor(
                            out=msg8[:, :half], in0=fe[:],
                            in1=hold["x4"][:, xo:xo + half], op=OP.mult)
                    else:
                        nc.vector.tensor_tensor(
                            out=msg8[:, :npair], in0=fsrc[:, :npair],
                            in1=hold["x4"][:, xo:xo + npair], op=OP.mult)
                    msgs[gp] = msg8

            def emit_mlp(wb):
                outw = outws.pop(wb)
                n = (min(wb * 4 + 4, W) - wb * 4) * P
                if DEBUG_DUMP and wb == 0:
                    dtile3 = cp.tile([C, 4 * P], f32, name="dtile3")
                    nc.vector.tensor_copy(out=dtile3[:, :n],
                                          in_=outw[:, :n])
                    nc.sync.dma_start(out=dbg_d[5, :, :n], in_=dtile3[:, :n])
                h1 = hp.tile([C, 4 * P], bf16, tag="h1")
                silu(h1[:, :n], outw[:, :n], b1_s)
                if DEBUG_DUMP and wb == 0:
                    dt6 = cp.tile([C, 4 * P], f32, name="dt6")
                    nc.vector.tensor_copy(out=dt6[:, :n], in_=h1[:, :n])
                    nc.sync.dma_start(out=dbg_d[6, :, :n], in_=dt6[:, :n])
                h2p = wps.tile([C, 4 * P], f32, space="PSUM",
                               name="outw")
                nc.tensor.matmul(out=h2p[:, :n], lhsT=w2t_s[:],
                                 rhs=h1[:, :n], start=True, stop=True)
                h2 = hp.tile([C, 4 * P], bf16, tag="h2")
                silu(h2[:, :n], h2p[:, :n], b2_s)
                if DEBUG_DUMP and wb == 0:
                    dt7 = cp.tile([C, 4 * P], f32, name="dt7")
                    nc.vector.tensor_copy(out=dt7[:, :n], in_=h2[:, :n])
                    nc.sync.dma_start(out=dbg_d[7, :, :n], in_=dt7[:, :n])
                    dt8 = cp.tile([C, 4 * P], f32, name="dt8")
                    nc.vector.tensor_copy(out=dt8[:, :n], in_=h2p[:, :n])
                    nc.sync.dma_start(out=dbg_d[8, :, :n], in_=dt8[:, :n])
                nc.tensor.matmul(out=h2p[0:1, :n], lhsT=w3t_s[:],
                                 rhs=h2[:, :n], start=True, stop=True)
                nc.scalar.activation(
                    ystrip[:, wb * 4 * P:wb * 4 * P + n], h2p[0:1, :n],
                    AF.Identity, bias=b3_s[:])
                nc.sync.dma_start(
                    out=out_d[None, wb * 4 * P:wb * 4 * P + n],
                    in_=ystrip[:, wb * 4 * P:wb * 4 * P + n])

            def emit_acc(gp):
                if gp not in msgs:
                    return
                msg8 = msgs.pop(gp)
                lo = gp * 2 * GC
                nreal = min(2 * GC, TTr - lo)
                j = 0
                while j < nreal:
                    w, k = sched[lo + j]
                    wb = w // 4
                    if wb not in outws:
                        outws[wb] = wps.tile([C, 4 * P], f32, space="PSUM",
                                             name="outw")
                    outw = outws[wb]
                    run = 1
                    while (run < 4 and j + run < nreal
                           and sched[lo + j + run][0] == w):
                        run += 1
                    klast = k + run - 1
                    reg = outw[:, (w % 4) * P:(w % 4 + 1) * P]
                    if FUSE_ACC and run > 1:
                        nc.tensor.matmul(
                            out=reg.unsqueeze(1).broadcast_to([C, run, P]),
                            lhsT=w1t_s[:],
                            rhs=msg8[:, j * P:(j + run) * P],
                            start=(k == 0),
                            stop=(klast == tiles_w[w] - 1))
                    else:
                        for q in range(run):
                            nc.tensor.matmul(
                                out=reg, lhsT=w1t_s[:],
                                rhs=msg8[:, (j + q) * P:(j + q + 1) * P],
                                start=(k + q == 0),
                                stop=(k + q == tiles_w[w] - 1))
                    if klast == tiles_w[w] - 1 and (w % 4 == 3 or w == W - 1):
                        emit_mlp(wb)
                    j += run

            for sb in range(NSB + 1):
                if sb < NSB:
                    emit_sb(sb)
                if sb >= 1:
                    for pr in range(2):
                        emit_acc((sb - 1) * 2 + pr)


    nc.compile()
    return nc


# --------------------------------------------------------------- driver

def run(inputs, trace=False, tmpdir=None):
    from concourse.bass_utils import run_bass_kernel_spmd

    in_maps, meta = prepare(**inputs)
    nc = build_graph(meta)
    res = run_bass_kernel_spmd(nc, in_maps, core_ids=list(range(NCORES)),
                               trace=trace, tmpdir=tmpdir)
    npc = meta["npc"]
    N = meta["N"]
    out = np.zeros(N, dtype=np.float32)
    for c in range(NCORES):
        out[meta["perms"][c]] = res.results[c]["out"][:npc]
    return out.reshape(N, 1), res


def kernel(**inputs):
    out, _ = run(inputs, trace=False)
    return out
